# revision 1
# baseline (speedup 1.0000x reference)
"""Trainium2 Bass kernel for 2-layer BaseRGCN (basis decomposition).

Design (8 NeuronCores, SPMD, dst-sharded aggregate-first):
  - Core c owns 49 dst windows of 128 nodes (6272 nodes; node space padded to
    50176, core 7's tail windows are partially dummy). Each core processes all
    edges whose dst falls in its range, so its output rows are complete — no
    AllReduce anywhere.
  - Aggregate-first: S_r[dst] = sum_{e: dst, et=r} norm_e * h[src_e] is built
    by gathering h rows (dma_gather, fp16) into slab tiles and accumulating
    per-(window, rel) one-hot matmuls into PSUM quarter-banks. The transform
    out[dst] = sum_r S_r @ W_r (+ rank-1 bias matmul) runs per window with
    S^T already feature-major in PSUM, producing [dst, feat] rows directly.
  - Edge buckets (window, rel, src-half) are FFD bin-packed into 128-slot
    columns: each bucket sits in exactly one column => exactly one matmul and
    one DVE one-hot (iota==drel)*norm per bucket, no boundary masking tax.
  - The only collective: AllGather of h1 (fp16) between layers, split into
    A (windows 0..23) and B (24..48) so A fires mid-layer-1. Gather index
    tables are shared between layers (same graph; layer 2's edge sort by
    etype in the reference is a permutation with identical sums).
  - fp16 end-to-end on device (PE 1 cycle/row, DVE 4x one-hot mode), fp32
    PSUM accumulation, fp32 final output. bias2 is added on host.
"""

import numpy as np

N_NODES = 50000
H_DIM = 128
NUM_RELS = 16
N_CORES = 8
P = 128
WPC = 49                    # windows per core (core 7: 48 real + 1 dummy)
NODES_PC = WPC * P          # 6272
A_W, B_W = 24, 25           # window split for the two h1 AllGathers
A_ROWS, B_ROWS = A_W * P, B_W * P        # 3072, 3200
TA_ROWS, TB_ROWS = N_CORES * A_ROWS, N_CORES * B_ROWS   # 24576, 25600
GROUP_W = 13                # windows per gather group
GROUPS = [(0, 13), (13, 13), (26, 13), (39, 10)]
MAX_GATHER_COLS = 8         # cols per dma_gather call (1024 idxs)

_CACHE = {}
_LAST_IN_MAPS = None


def _build_edge_org(src, dst, etype, norm):
    """Host-side edge organization.

    Returns (prog, per_core) where prog is identical across cores:
      prog['qcols'][g][q]    columns in (group g, src-half q) gather run
      prog['buckets']        list over emission order (lw, rel) of
                             [(col, n_idx_pair...)] entries: each bucket is
                             (lw, rel, q, col, off, cnt, pair_idx)
    and per_core has gidx (int16 gather idxs), drel/norm pair tables.
    """
    E = src.shape[0]
    c = np.minimum(dst // NODES_PC, N_CORES - 1)
    local = dst - c * NODES_PC
    lw = local // P
    drel = (local % P).astype(np.float32)

    sc = np.minimum(src // NODES_PC, N_CORES - 1)
    sl = src - sc * NODES_PC
    q = (sl >= A_ROWS).astype(np.int64)
    row = np.where(q == 1, sc * B_ROWS + (sl - A_ROWS), sc * A_ROWS + sl)

    grp_of = np.zeros(WPC, np.int64)
    for g, (w0, nw) in enumerate(GROUPS):
        grp_of[w0:w0 + nw] = g

    # counts per (core, lw, rel, q) -> mx over cores
    counts = np.zeros((N_CORES, WPC, NUM_RELS, 2), np.int64)
    np.add.at(counts, (c, lw, etype, q), 1)
    mx = counts.max(axis=0)          # [WPC, R, 2]
    mx = np.maximum(mx, 1)           # every bucket gets >= 1 slot

    # FFD bin-pack per (group, q): items (lw, rel) with size mx[lw, rel, q]
    # (split items > 128 into chunks). Column capacity 128.
    col_base = {}        # (g, q) -> global col start
    qcols = [[0, 0] for _ in GROUPS]
    buckets = {}         # (lw, rel, q) -> list of (gcol, off, cnt)
    gcol = 0
    for g, (w0, nw) in enumerate(GROUPS):
        for qq in (0, 1):
            items = []
            for w in range(w0, w0 + nw):
                for r in range(NUM_RELS):
                    size = int(mx[w, r, qq])
                    while size > P:
                        items.append((P, w, r))
                        size -= P
                    if size > 0:
                        items.append((size, w, r))
            items.sort(key=lambda t: -t[0])
            cols_fill = []   # per local col: used slots
            placed = []      # (local_col, off, cnt, w, r)
            for size, w, r in items:
                for ci in range(len(cols_fill)):
                    if cols_fill[ci] + size <= P:
                        placed.append((ci, cols_fill[ci], size, w, r))
                        cols_fill[ci] += size
                        break
                else:
                    placed.append((len(cols_fill), 0, size, w, r))
                    cols_fill.append(size)
            col_base[(g, qq)] = gcol
            qcols[g][qq] = len(cols_fill)
            for ci, off, size, w, r in placed:
                buckets.setdefault((w, r, qq), []).append(
                    (gcol + ci, off, size))
            gcol += len(cols_fill)
    tot_cols = gcol
    tot_slots = tot_cols * P

    # pair ordering: emission order (lw, rel, q, chunk)
    prog_buckets = []    # (lw, rel) -> list of (gcol, off, cnt, pair)
    pair = 0
    emit = {}
    for w in range(WPC):
        for r in range(NUM_RELS):
            lst = []
            for qq in (0, 1):
                for (gc, off, cnt) in buckets.get((w, r, qq), []):
                    lst.append((gc, off, cnt, pair, qq))
                    pair += 1
            emit[(w, r)] = lst
            prog_buckets.append(lst)
    n_pairs = pair

    # per-core slot fill
    bucket_id = lw * (NUM_RELS * 2) + etype * 2 + q     # fine bucket key
    # slot start per (bucket chunk): build arrays bucket -> chunk list
    gidx = np.zeros((N_CORES, tot_slots), np.int16)
    drel_pair = np.full((N_CORES, P, n_pairs), -1.0, np.float32)
    norm_pair = np.zeros((N_CORES, P, n_pairs), np.float32)

    # rank within bucket per edge, per core
    for cc in range(N_CORES):
        m = c == cc
        eb = bucket_id[m]
        er = row[m]
        ed = drel[m]
        en = norm[m, 0]
        order = np.argsort(eb, kind="stable")
        eb_s, er_s, ed_s, en_s = eb[order], er[order], ed[order], en[order]
        # rank within each bucket
        uniq, start_idx, cnts = np.unique(
            eb_s, return_index=True, return_counts=True)
        rank = np.arange(eb_s.shape[0]) - np.repeat(start_idx, cnts)
        # map (bucket, rank) -> slot + pair
        # build per bucket chunk tables
        for bi, s0, cn in zip(uniq, start_idx, cnts):
            w = int(bi) // (NUM_RELS * 2)
            r = (int(bi) // 2) % NUM_RELS
            qq = int(bi) % 2
            chunks = [e for e in emit[(w, r)] if e[4] == qq]
            pos = 0
            for (gc, off, csz, pj, _) in chunks:
                take = min(csz, cn - pos)
                if take <= 0:
                    break
                sel = slice(s0 + pos, s0 + pos + take)
                slots = gc * P + off + np.arange(take)
                gidx[cc, slots] = er_s[sel].astype(np.int16)
                pp = (off + np.arange(take))
                drel_pair[cc, pp, pj] = ed_s[sel]
                norm_pair[cc, pp, pj] = en_s[sel]
                pos += take
            assert pos == cn, f"bucket overflow {w},{r},{qq}: {cn} vs {pos}"

    # wrap gather idxs: [128, tot_slots/16]
    gidx2 = np.zeros((N_CORES, 128, tot_slots // 16), np.int16)
    for cc in range(N_CORES):
        a = gidx[cc].reshape(tot_slots // 16, 16).T
        gidx2[cc] = np.tile(a, (8, 1))

    prog = dict(qcols=qcols, col_base=col_base, emit=emit,
                tot_cols=tot_cols, n_pairs=n_pairs)
    return prog, gidx2, drel_pair, norm_pair


def _build_bass(prog, repeat=1, stage=3):
    from concourse import bacc, bass, mybir, tile

    f16 = mybir.dt.float16
    f32 = mybir.dt.float32
    i16 = mybir.dt.int16
    nc = bacc.Bacc("TRN2", target_bir_lowering=False, debug=False,
                   num_devices=N_CORES, dynamic_dma_scratch_size=32768)

    tot_cols = prog["tot_cols"]
    n_pairs = prog["n_pairs"]
    qcols = prog["qcols"]
    col_base = prog["col_base"]
    emit = prog["emit"]

    hA = nc.dram_tensor("hA", [TA_ROWS, P], f16, kind="ExternalInput")
    hB = nc.dram_tensor("hB", [TB_ROWS, P], f16, kind="ExternalInput")
    W1 = nc.dram_tensor("W1", [P, NUM_RELS * P], f16, kind="ExternalInput")
    W2 = nc.dram_tensor("W2", [P, NUM_RELS * P], f16, kind="ExternalInput")
    iota = nc.dram_tensor("iota", [P, P], f16, kind="ExternalInput")
    b1rep = nc.dram_tensor("b1rep", [P, P], f16, kind="ExternalInput")
    drel = nc.dram_tensor("drel", [P, n_pairs], f32, kind="ExternalInput")
    normv = nc.dram_tensor("normv", [P, n_pairs], f32, kind="ExternalInput")
    gidx = nc.dram_tensor("gidx", [P, tot_cols * 8], i16, kind="ExternalInput")

    h1kind = {"kind": "ExternalOutput"} if stage == 1 else {}
    h1ownA = nc.dram_tensor("h1ownA", [A_ROWS, P], f16, **h1kind)
    h1ownB = nc.dram_tensor("h1ownB", [B_ROWS, P], f16, **h1kind)
    tabA = nc.dram_tensor("tabA", [TA_ROWS, P], f16, addr_space="Shared")
    tabB = nc.dram_tensor("tabB", [TB_ROWS, P], f16, addr_space="Shared")
    out = nc.dram_tensor("out", [NODES_PC, P], f32, kind="ExternalOutput")

    GMAX = max(qc[0] + qc[1] for qc in qcols)
    rg = [list(range(N_CORES))]

    with tile.TileContext(nc) as tc:
        with tc.tile_pool(name="const", bufs=1) as constp:
            iota_t = constp.tile([P, P], f16)
            nc.sync.dma_start(out=iota_t[:], in_=iota[:])
            b1_t = constp.tile([P, P], f16)
            nc.sync.dma_start(out=b1_t[:], in_=b1rep[:])
            ones_t = constp.tile([P, P], f16)
            nc.vector.memset(ones_t[:], 1.0)
            drel_t = constp.tile([P, n_pairs], f32)
            nc.sync.dma_start(out=drel_t[:], in_=drel[:])
            norm_t = constp.tile([P, n_pairs], f32)
            nc.sync.dma_start(out=norm_t[:], in_=normv[:])
            gidx_t = constp.tile([P, tot_cols * 8], i16)
            nc.sync.dma_start(out=gidx_t[:], in_=gidx[:])
            w_t = [constp.tile([P, NUM_RELS * P], f16, name=f"w{i}")
                   for i in (0, 1)]
            nc.sync.dma_start(out=w_t[0][:], in_=W1[:])
            nc.sync.dma_start(out=w_t[1][:], in_=W2[:])

            def layer(l):
                src_tabs = (hA, hB) if l == 0 else (tabA, tabB)
                with tc.tile_pool(name=f"slab{l}", bufs=2) as slabp, \
                     tc.tile_pool(name=f"oh{l}", bufs=24) as ohp, \
                     tc.tile_pool(name=f"S{l}", bufs=3, space="PSUM") as psS, \
                     tc.tile_pool(name=f"O{l}", bufs=2, space="PSUM") as psO, \
                     tc.tile_pool(name=f"ssb{l}", bufs=10) as ssbp, \
                     tc.tile_pool(name=f"hsb{l}", bufs=3) as hsbp:
                    slabs = {}
                    obank = [None]
                    pend_tf = []      # (lw, [Ssb x4])
                    pend_flush = []   # (lw, quarter emitted) windows in obank

                    def do_gather(g):
                        slab = slabp.tile([P, GMAX * P], f16, tag="slab")
                        sl3 = slab[:].rearrange("p (t d) -> p t d", t=GMAX)
                        lc0 = 0
                        for qq in (0, 1):
                            ncols = qcols[g][qq]
                            base = col_base[(g, qq)]
                            done = 0
                            while done < ncols:
                                piece = min(MAX_GATHER_COLS, ncols - done)
                                nidx = piece * P
                                g0 = (base + done) * 8
                                nc.gpsimd.dma_gather(
                                    out_ap=sl3[:, lc0 + done:lc0 + done + piece, :],
                                    in_ap=src_tabs[qq][:, :],
                                    idxs_ap=gidx_t[:, g0:g0 + nidx // 16],
                                    num_idxs=nidx,
                                    num_idxs_reg=nidx,
                                    elem_size=P,
                                )
                                done += piece
                            lc0 += ncols
                        slabs[g] = slab

                    def emit_transform(lw, ssb4):
                        k = lw % 4
                        if k == 0:
                            obank[0] = psO.tile([P, 4 * P], f32, space="PSUM",
                                                tag="ob", name="ob")
                        quarter = obank[0][:, k * P:(k + 1) * P]
                        for r in range(NUM_RELS):
                            nc.tensor.matmul(
                                out=quarter,
                                lhsT=ssb4[r // 4][:, (r % 4) * P:(r % 4 + 1) * P],
                                rhs=w_t[l][:, r * P:(r + 1) * P],
                                start=(r == 0), stop=(r == NUM_RELS - 1 and l != 0))
                        if l == 0:
                            nc.tensor.matmul(out=quarter, lhsT=ones_t[:],
                                             rhs=b1_t[:], start=False, stop=True)
                        pend_flush.append(lw)
                        if k == 3 or lw == WPC - 1:
                            flush()

                    def flush():
                        nwin = len(pend_flush)
                        w0 = pend_flush[0]
                        hs = hsbp.tile([P, 4 * P], f16 if l == 0 else f32,
                                       tag="hs")
                        nc.scalar.activation(
                            out=hs[:, :nwin * P], in_=obank[0][:, :nwin * P],
                            func=(mybir.ActivationFunctionType.Relu if l == 0
                                  else mybir.ActivationFunctionType.Copy))
                        for k, lw in enumerate(pend_flush):
                            if l == 0:
                                if lw < A_W:
                                    dst_ap = h1ownA[lw * P:(lw + 1) * P, :]
                                else:
                                    dst_ap = h1ownB[(lw - A_W) * P:
                                                    (lw - A_W + 1) * P, :]
                            else:
                                dst_ap = out[lw * P:(lw + 1) * P, :]
                            nc.sync.dma_start(out=dst_ap,
                                              in_=hs[:, k * P:(k + 1) * P])
                        pend_flush.clear()
                        if l == 0 and stage >= 2:
                            if w0 + nwin == A_W:
                                nc.gpsimd.collective_compute(
                                    "AllGather", mybir.AluOpType.bypass,
                                    ins=[h1ownA.ap().opt()],
                                    outs=[tabA.ap().opt()],
                                    replica_groups=rg)
                            if w0 + nwin == WPC:
                                nc.gpsimd.collective_compute(
                                    "AllGather", mybir.AluOpType.bypass,
                                    ins=[h1ownB.ap().opt()],
                                    outs=[tabB.ap().opt()],
                                    replica_groups=rg)

                    do_gather(0)
                    for g, (gw0, gnw) in enumerate(GROUPS):
                        if g + 1 < len(GROUPS):
                            do_gather(g + 1)
                        slab = slabs[g]
                        gbase = col_base[(g, 0)]
                        for lw in range(gw0, gw0 + gnw):
                            ssb4 = []
                            for bg in range(4):
                                S = psS.tile([P, 4 * P], f32,
                                             space="PSUM", tag="S")
                                for rl in range(4):
                                    r = bg * 4 + rl
                                    quarter = S[:, rl * P:(rl + 1) * P]
                                    lst = emit[(lw, r)]
                                    for k, (gc, off, cnt, pj, qq) in enumerate(lst):
                                        oh = ohp.tile([P, P], f16, tag="oh")
                                        nc.vector.tensor_scalar(
                                            out=oh[:], in0=iota_t[:],
                                            scalar1=drel_t[:, pj:pj + 1],
                                            scalar2=norm_t[:, pj:pj + 1],
                                            op0=mybir.AluOpType.is_equal,
                                            op1=mybir.AluOpType.mult)
                                        lc = gc - gbase
                                        nc.tensor.matmul(
                                            out=quarter,
                                            lhsT=slab[:, lc * P:(lc + 1) * P],
                                            rhs=oh[:],
                                            start=(k == 0),
                                            stop=(k == len(lst) - 1))
                                ssb = ssbp.tile([P, 4 * P], f16, tag="ssb")
                                nc.scalar.activation(
                                    out=ssb[:], in_=S[:],
                                    func=mybir.ActivationFunctionType.Copy)
                                ssb4.append(ssb)
                            pend_tf.append((lw, ssb4))
                            if len(pend_tf) > 1:
                                emit_transform(*pend_tf.pop(0))
                        del slabs[g]
                    while pend_tf:
                        emit_transform(*pend_tf.pop(0))

            for _ in range(repeat):
                layer(0)
                if stage >= 2:
                    layer(1)

    nc.compile()
    return nc


def _host_prep(h, norm, src, dst, etype, V1, coef1, bias1, V2, coef2):
    W1 = np.einsum("rb,bio->rio", np.asarray(coef1, np.float64),
                   np.asarray(V1, np.float64)).astype(np.float32)
    W2 = np.einsum("rb,bio->rio", np.asarray(coef2, np.float64),
                   np.asarray(V2, np.float64)).astype(np.float32)
    # [fi, rel*fo] layout
    W1t = np.concatenate([W1[r] for r in range(NUM_RELS)], axis=1)
    W2t = np.concatenate([W2[r] for r in range(NUM_RELS)], axis=1)

    # h permuted into (A|B) table layout
    hp = np.zeros((N_CORES * NODES_PC, H_DIM), np.float32)
    hp[:N_NODES] = np.asarray(h, np.float32)
    hp = hp.reshape(N_CORES, NODES_PC, H_DIM)
    hA = np.ascontiguousarray(hp[:, :A_ROWS].reshape(TA_ROWS, H_DIM))
    hB = np.ascontiguousarray(hp[:, A_ROWS:].reshape(TB_ROWS, H_DIM))

    iota_np = np.broadcast_to(np.arange(P, dtype=np.float32), (P, P)).copy()
    b1rep = np.broadcast_to(
        np.asarray(bias1, np.float32) / P, (P, P)).copy()
    return (W1t.astype(np.float16), W2t.astype(np.float16),
            hA.astype(np.float16), hB.astype(np.float16),
            iota_np.astype(np.float16), b1rep.astype(np.float16))


def build(src, dst, etype, norm, repeat=1, stage=3):
    prog, gidx2, drel_pair, norm_pair = _build_edge_org(
        np.asarray(src), np.asarray(dst), np.asarray(etype), np.asarray(norm))
    nc = _build_bass(prog, repeat=repeat, stage=stage)
    return nc, prog, gidx2, drel_pair, norm_pair


def kernel(h, norm, src, dst, etype, V1, coef1, bias1, V2, coef2, bias2,
           stage=3):
    import hashlib
    key = hashlib.md5(
        np.asarray(src).tobytes() + np.asarray(dst).tobytes()
        + np.asarray(etype).tobytes()
    ).hexdigest() + f"s{stage}"
    if key not in _CACHE:
        _CACHE[key] = build(src, dst, etype, norm, stage=stage)
    nc, prog, gidx2, drel_pair, norm_pair = _CACHE[key]

    W1t, W2t, hA, hB, iota_np, b1rep = _host_prep(
        h, norm, src, dst, etype, V1, coef1, bias1, V2, coef2)

    in_maps = []
    for c in range(N_CORES):
        in_maps.append({
            "hA": hA, "hB": hB, "W1": W1t, "W2": W2t,
            "iota": iota_np, "b1rep": b1rep,
            "drel": drel_pair[c].astype(np.float32),
            "normv": norm_pair[c].astype(np.float32),
            "gidx": gidx2[c],
        })

    global _LAST_IN_MAPS
    _LAST_IN_MAPS = in_maps
    from concourse.bass_utils import run_bass_kernel_spmd
    res = run_bass_kernel_spmd(nc, in_maps, core_ids=list(range(N_CORES)))

    if stage == 1:
        h1 = np.concatenate(
            [np.concatenate([res.results[c]["h1ownA"],
                             res.results[c]["h1ownB"]], axis=0)
             for c in range(N_CORES)], axis=0)
        return h1[:N_NODES].astype(np.float32)

    outs = np.concatenate([res.results[c]["out"] for c in range(N_CORES)],
                          axis=0)
    return (outs[:N_NODES]
            + np.asarray(bias2, np.float32)[None, :]).astype(np.float32)

